# revision 28
# baseline (speedup 1.0000x reference)
"""Block self-attention (Gaussian kernel weights) Trainium2 Bass kernel.

For each independent block of B=1024 rows of `features` [262144, 128]:
    sq_i = ||x_i||^2 ;  d2 = sq_i + sq_j - 2 x@x^T ;  w = exp(-max(d2,0)/25.6)
    out  = (w @ x) / B
Blocks are data-parallel across 8 NeuronCores (32 blocks per core).

Numerics: matmul operands are bf16, but the diagonal (w_ii = 1 exactly; it
dominates out — off-diagonal mass is only ~0.8%) is excluded from the matmul
(A's diagonal is zeroed on GPSIMD) and re-added as x/B in full fp32 at the
end.  Algorithm error vs the fp32 reference: rel-L2 ~4e-5.

Per-block schedule (c = 8 row-chunks of 128 rows):
    prologue: DMA x (fp32) + cast-DMA x -> xr (bf16, SWDGE);
              xsq = xr*xr (GPSIMD), bias_c = -sum(xsq)/25.6 (DVE reduce+scale);
              e/B = exp(bias + ln(1/B)) (ScalarE);
              8 DMA-xbar transposes xr -> xT [d, j] bf16
    c-loop:   G_c = xT[:,c].T @ xT              (2x N=512 bf16 matmuls -> fp32 PSUM)
              A_c = exp(G_c*2/25.6 + bias_c)    (ScalarE -> bf16, per-part bias = e_j)
              diag(A_c) = 0                     (GPSIMD affine_select)
              outT += xr_c.T @ A_c              (2x N=512 matmuls, PSUM accumulate)
    epilogue: outT -> bf16 SBUF (DVE casts), 8 DMA-xbar transposes -> [i, d],
              tmp = trd * (e_i/B)               (DVE broadcast multiply)
              out = x*(1/B) + tmp               (DVE scalar_tensor_tensor)
              DMA out (fp32)
"""

import math
import os

# Recover wedged NeuronCores from any previously crashed process.
os.environ.setdefault("NEURON_RT_RESET_CORES", "1")

import numpy as np

import concourse.bass as bass
import concourse.tile as tile
from concourse import bacc, mybir
from concourse.bass_utils import run_bass_kernel_spmd
from concourse.masks import make_identity

N_TOTAL = 262144
D = 128
B = 1024
NCORES = 8
ROWS_PER_CORE = N_TOTAL // NCORES   # 32768
NB_FULL = ROWS_PER_CORE // B        # 32 blocks per core
C = B // 128                        # 8 row-chunks per block

F32 = mybir.dt.float32
BF16 = mybir.dt.bfloat16

SIGMA2X2 = 2.0 * (D / 10.0)         # 25.6
G_SCALE = 2.0 / SIGMA2X2            # 0.078125
NEG_INV = -1.0 / SIGMA2X2           # -0.0390625

EXP = mybir.ActivationFunctionType.Exp
ADD = mybir.AluOpType.add
MULT = mybir.AluOpType.mult
NE = mybir.AluOpType.not_equal


def build(nb: int = NB_FULL) -> bacc.Bacc:
    rows = nb * B
    nc = bacc.Bacc("TRN2", target_bir_lowering=False, debug=False)

    fin = nc.dram_tensor("features", [rows, D], F32, kind="ExternalInput").ap()
    fout = nc.dram_tensor("out", [rows, D], F32, kind="ExternalOutput").ap()

    # [b, p, c, d]: row index = b*1024 + c*128 + p
    fin_v = fin.rearrange("(b c p) d -> b p c d", p=128, c=C)
    fout_v = fout.rearrange("(b c p) d -> b p c d", p=128, c=C)

    with tile.TileContext(nc) as tc:
        with (
            tc.tile_pool(name="const", bufs=1) as cpool,
            tc.tile_pool(name="x", bufs=3) as xpool,
            tc.tile_pool(name="xr", bufs=3) as xrpool,
            tc.tile_pool(name="xt", bufs=2) as xtpool,
            tc.tile_pool(name="sq", bufs=3) as sqpool,
            tc.tile_pool(name="jk", bufs=2) as jkpool,
            tc.tile_pool(name="a", bufs=4) as apool,
            tc.tile_pool(name="ot", bufs=2) as otpool,
            tc.tile_pool(name="tmp", bufs=2) as tmppool,
            tc.tile_pool(name="osb", bufs=2) as opool,
            tc.tile_pool(name="gps", bufs=2, space="PSUM") as gpool,
            tc.tile_pool(name="acc", bufs=2, space="PSUM") as accpool,
            tc.tile_pool(name="trp", bufs=2, space="PSUM") as trpool,
        ):
            lnb = cpool.tile([128, 1], F32)
            nc.gpsimd.memset(lnb[:], math.log(1.0 / B))
            identb = cpool.tile([128, 128], BF16)
            make_identity(nc, identb[:])
            # maskbar: 1 everywhere except 0 on the diagonal
            maskbar = cpool.tile([128, 128], BF16)
            nc.gpsimd.memset(maskbar[:], 1.0)
            nc.gpsimd.affine_select(
                out=maskbar[:], in_=maskbar[:], pattern=[[-1, 128]],
                compare_op=NE, fill=0.0, base=0, channel_multiplier=1,
            )

            state: dict[int, dict] = {}
            LAG = 2  # chunks mm2 trails mm1 by, hiding the exp+diag chain

            def stage_load(b: int):
                """DMA in (fp32 + bf16 cast) and the sq/bias/e chain."""
                x_sb = xpool.tile([128, C, D], F32)
                nc.sync.dma_start(out=x_sb[:], in_=fin_v[b])
                xr = xrpool.tile([128, C, D], BF16)
                nc.gpsimd.dma_start(out=xr[:], in_=fin_v[b])  # SWDGE cast DMA

                xr_flat = xr[:].rearrange("p c d -> p (c d)")
                xsq = jkpool.tile([128, C * D], F32)
                nc.gpsimd.tensor_mul(xsq[:], xr_flat, xr_flat)
                sqcol = sqpool.tile([128, C], F32)
                nc.vector.tensor_reduce(
                    sqcol[:], xsq[:].rearrange("p (c d) -> p c d", d=D),
                    axis=mybir.AxisListType.X, op=ADD,
                )
                bias_col = sqpool.tile([128, C], F32)
                nc.vector.tensor_scalar_mul(bias_col[:], sqcol[:], NEG_INV)
                escale = sqpool.tile([128, C], F32)  # e_i/B
                nc.scalar.activation(escale[:], bias_col[:], EXP, bias=lnb[:])

                state[b] = dict(
                    x_sb=x_sb, xr=xr, bias_col=bias_col, escale=escale
                )

            def trans_in(b: int, c: int):
                st = state[b]
                if c == 0:
                    trt = trpool.tile([128, C, D], BF16, tag="trt")
                    st["trt"] = trt
                nc.tensor.transpose(
                    out=st["trt"][:, c, :], in_=st["xr"][:, c, :],
                    identity=identb[:],
                )

            def xt_copy(b: int):
                st = state[b]
                xT = xtpool.tile([128, B], BF16)
                nc.vector.tensor_copy(
                    xT[:], st.pop("trt")[:].rearrange("p c d -> p (c d)")
                )
                st["xT"] = xT

            def mm1_exp(b: int, c: int):
                st = state[b]
                if c == 0:
                    o0 = accpool.tile([128, 512], F32, tag="outT")
                    o1 = accpool.tile([128, 512], F32, tag="outT")
                    st["outT"] = [o0, o1]
                    st["a_tiles"] = {}
                xT, bias_col = st["xT"], st["bias_col"]
                g = gpool.tile([128, B], F32)
                for h in range(2):
                    nc.tensor.matmul(
                        g[:, h * 512:(h + 1) * 512],
                        lhsT=xT[:, c * 128:(c + 1) * 128],
                        rhs=xT[:, h * 512:(h + 1) * 512],
                        start=True, stop=True,
                    )
                a_c = apool.tile([128, B], BF16)
                nc.scalar.activation(
                    a_c[:], g[:], EXP,
                    bias=bias_col[:, c:c + 1], scale=G_SCALE,
                )
                # zero the diagonal of the c-th 128x128 sub-block
                nc.vector.tensor_mul(
                    a_c[:, c * 128:(c + 1) * 128],
                    a_c[:, c * 128:(c + 1) * 128],
                    maskbar[:],
                )
                st["a_tiles"][c] = a_c

            def mm2(b: int, c: int):
                st = state[b]
                a_c = st["a_tiles"].pop(c)
                for h in range(2):
                    nc.tensor.matmul(
                        st["outT"][h][:],
                        lhsT=st["xr"][:, c, :],
                        rhs=a_c[:, h * 512:(h + 1) * 512],
                        start=(c == 0), stop=(c == C - 1),
                    )

            def casts(b: int):
                st = state[b]
                outT_sb = otpool.tile([128, B], BF16)
                for h in range(2):
                    nc.vector.tensor_copy(
                        outT_sb[:, h * 512:(h + 1) * 512], st["outT"][h][:]
                    )
                st["outT_sb"] = outT_sb

            def trans_out(b: int, c: int):
                st = state[b]
                if c == 0:
                    trt2 = trpool.tile([128, C, D], BF16, tag="trt")
                    st["trt2"] = trt2
                nc.tensor.transpose(
                    out=st["trt2"][:, c, :],
                    in_=st["outT_sb"][:, c * 128:(c + 1) * 128],
                    identity=identb[:],
                )

            def tail(b: int):
                st = state.pop(b)
                tmp = tmppool.tile([128, C, D], F32)
                nc.vector.tensor_mul(
                    tmp[:], st["trt2"][:],
                    st["escale"][:].unsqueeze(2).broadcast_to([128, C, D]),
                )
                out_final = opool.tile([128, C, D], F32)
                nc.vector.scalar_tensor_tensor(
                    out=out_final[:], in0=st["x_sb"][:], scalar=1.0 / B,
                    in1=tmp[:], op0=MULT, op1=ADD,
                )
                nc.sync.dma_start(out=fout_v[b], in_=out_final[:])

            # 3-deep software pipeline: while block b's matmul loop runs,
            # block b+1's in-transposes and block b-1's out-transposes fill
            # the PE gaps left by waiting on ScalarE's exp.
            stage_load(0)
            if nb > 1:
                stage_load(1)
            for c in range(C):
                trans_in(0, c)
            xt_copy(0)
            for b in range(nb):
                if b >= 1:
                    casts(b - 1)
                for c in range(C + LAG):
                    if c < C:
                        mm1_exp(b, c)
                        if b + 1 < nb:
                            trans_in(b + 1, c)
                        if b >= 1:
                            trans_out(b - 1, c)
                    if c >= LAG:
                        mm2(b, c - LAG)
                if b + 1 < nb:
                    xt_copy(b + 1)
                if b >= 1:
                    tail(b - 1)
                if b + 2 < nb:
                    stage_load(b + 2)
            casts(nb - 1)
            for c in range(C):
                trans_out(nb - 1, c)
            tail(nb - 1)

    nc.compile()
    return nc


_CACHE: dict[int, bacc.Bacc] = {}


def _get_nc(nb: int = NB_FULL) -> bacc.Bacc:
    if nb not in _CACHE:
        _CACHE[nb] = build(nb)
    return _CACHE[nb]


def run(features: np.ndarray, nc: bacc.Bacc | None = None, **spmd_kwargs):
    """Shard rows across 8 cores, run, gather. Returns (out, BassKernelResults)."""
    features = np.ascontiguousarray(features, dtype=np.float32)
    assert features.shape == (N_TOTAL, D)
    if nc is None:
        nc = _get_nc()
    core_ids = list(range(NCORES))
    shards = np.split(features, NCORES, axis=0)
    in_maps = [{"features": s} for s in shards]
    res = run_bass_kernel_spmd(nc, in_maps, core_ids, **spmd_kwargs)
    out = np.concatenate([res.results[i]["out"] for i in range(NCORES)], axis=0)
    return out, res


def kernel(features: np.ndarray) -> np.ndarray:
    out, _ = run(features)
    return out


# revision 29
# speedup vs baseline: 1.1942x; 1.1942x over previous
"""Block self-attention (Gaussian kernel weights) Trainium2 Bass kernel.

For each independent block of B=1024 rows of `features` [262144, 128]:
    sq_i = ||x_i||^2 ;  d2 = sq_i + sq_j - 2 x@x^T ;  w = exp(-max(d2,0)/25.6)
    out  = (w @ x) / B
Blocks are data-parallel across 8 NeuronCores (32 blocks per core).

Numerics: matmul operands are bf16, but the diagonal (w_ii = 1 exactly; it
dominates out — off-diagonal mass is only ~0.8%) is excluded from the matmul
(A's diagonal is zeroed on GPSIMD) and re-added as x/B in full fp32 at the
end.  Algorithm error vs the fp32 reference: rel-L2 ~4e-5.

Per-block schedule (c = 8 row-chunks of 128 rows):
    prologue: DMA x (fp32) + cast-DMA x -> xr (bf16, SWDGE);
              xsq = xr*xr (GPSIMD), bias_c = -sum(xsq)/25.6 (DVE reduce+scale);
              e/B = exp(bias + ln(1/B)) (ScalarE);
              8 DMA-xbar transposes xr -> xT [d, j] bf16
    c-loop:   G_c = xT[:,c].T @ xT              (2x N=512 bf16 matmuls -> fp32 PSUM)
              A_c = exp(G_c*2/25.6 + bias_c)    (ScalarE -> bf16, per-part bias = e_j)
              diag(A_c) = 0                     (GPSIMD affine_select)
              outT += xr_c.T @ A_c              (2x N=512 matmuls, PSUM accumulate)
    epilogue: outT -> bf16 SBUF (DVE casts), 8 DMA-xbar transposes -> [i, d],
              tmp = trd * (e_i/B)               (DVE broadcast multiply)
              out = x*(1/B) + tmp               (DVE scalar_tensor_tensor)
              DMA out (fp32)
"""

import math
import os

# Recover wedged NeuronCores from any previously crashed process.
os.environ.setdefault("NEURON_RT_RESET_CORES", "1")

import numpy as np

import concourse.bass as bass
import concourse.tile as tile
from concourse import bacc, mybir
from concourse.bass_utils import run_bass_kernel_spmd
from concourse.masks import make_identity

N_TOTAL = 262144
D = 128
B = 1024
NCORES = 8
ROWS_PER_CORE = N_TOTAL // NCORES   # 32768
NB_FULL = ROWS_PER_CORE // B        # 32 blocks per core
C = B // 128                        # 8 row-chunks per block

F32 = mybir.dt.float32
BF16 = mybir.dt.bfloat16

SIGMA2X2 = 2.0 * (D / 10.0)         # 25.6
G_SCALE = 2.0 / SIGMA2X2            # 0.078125
NEG_INV = -1.0 / SIGMA2X2           # -0.0390625

EXP = mybir.ActivationFunctionType.Exp
ADD = mybir.AluOpType.add
MULT = mybir.AluOpType.mult
NE = mybir.AluOpType.not_equal


def build(nb: int = NB_FULL) -> bacc.Bacc:
    rows = nb * B
    nc = bacc.Bacc("TRN2", target_bir_lowering=False, debug=False)

    fin = nc.dram_tensor("features", [rows, D], F32, kind="ExternalInput").ap()
    fout = nc.dram_tensor("out", [rows, D], F32, kind="ExternalOutput").ap()

    # [b, p, c, d]: row index = b*1024 + c*128 + p
    fin_v = fin.rearrange("(b c p) d -> b p c d", p=128, c=C)
    fout_v = fout.rearrange("(b c p) d -> b p c d", p=128, c=C)

    with tile.TileContext(nc) as tc:
        with (
            tc.tile_pool(name="const", bufs=1) as cpool,
            tc.tile_pool(name="x", bufs=3) as xpool,
            tc.tile_pool(name="xr", bufs=3) as xrpool,
            tc.tile_pool(name="xt", bufs=2) as xtpool,
            tc.tile_pool(name="sq", bufs=3) as sqpool,
            tc.tile_pool(name="jk", bufs=2) as jkpool,
            tc.tile_pool(name="a", bufs=6) as apool,
            tc.tile_pool(name="ot", bufs=2) as otpool,
            tc.tile_pool(name="tmp", bufs=2) as tmppool,
            tc.tile_pool(name="osb", bufs=2) as opool,
            tc.tile_pool(name="gps", bufs=2, space="PSUM") as gpool,
            tc.tile_pool(name="acc", bufs=2, space="PSUM") as accpool,
            tc.tile_pool(name="trp", bufs=2, space="PSUM") as trpool,
        ):
            lnb = cpool.tile([128, 1], F32)
            nc.gpsimd.memset(lnb[:], math.log(1.0 / B))
            identb = cpool.tile([128, 128], BF16)
            make_identity(nc, identb[:])
            # maskbar: 1 everywhere except 0 on the diagonal
            maskbar = cpool.tile([128, 128], BF16)
            nc.gpsimd.memset(maskbar[:], 1.0)
            nc.gpsimd.affine_select(
                out=maskbar[:], in_=maskbar[:], pattern=[[-1, 128]],
                compare_op=NE, fill=0.0, base=0, channel_multiplier=1,
            )

            state: dict[int, dict] = {}
            LAG = 3  # chunks mm2 trails mm1 by, hiding the exp+diag chain

            def stage_load(b: int):
                """DMA in (fp32 + bf16 cast) and the sq/bias/e chain."""
                x_sb = xpool.tile([128, C, D], F32)
                nc.sync.dma_start(out=x_sb[:], in_=fin_v[b])
                xr = xrpool.tile([128, C, D], BF16)
                nc.gpsimd.dma_start(out=xr[:], in_=fin_v[b])  # SWDGE cast DMA

                xr_flat = xr[:].rearrange("p c d -> p (c d)")
                xsq = jkpool.tile([128, C * D], F32)
                nc.gpsimd.tensor_mul(xsq[:], xr_flat, xr_flat)
                sqcol = sqpool.tile([128, C], F32)
                nc.vector.tensor_reduce(
                    sqcol[:], xsq[:].rearrange("p (c d) -> p c d", d=D),
                    axis=mybir.AxisListType.X, op=ADD,
                )
                bias_col = sqpool.tile([128, C], F32)
                nc.vector.tensor_scalar_mul(bias_col[:], sqcol[:], NEG_INV)
                escale = sqpool.tile([128, C], F32)  # e_i/B
                nc.scalar.activation(escale[:], bias_col[:], EXP, bias=lnb[:])

                state[b] = dict(
                    x_sb=x_sb, xr=xr, bias_col=bias_col, escale=escale
                )

            def trans_in(b: int, c: int):
                st = state[b]
                if c == 0:
                    trt = trpool.tile([128, C, D], BF16, tag="trt")
                    st["trt"] = trt
                nc.tensor.transpose(
                    out=st["trt"][:, c, :], in_=st["xr"][:, c, :],
                    identity=identb[:],
                )

            def xt_copy(b: int):
                st = state[b]
                xT = xtpool.tile([128, B], BF16)
                nc.vector.tensor_copy(
                    xT[:], st.pop("trt")[:].rearrange("p c d -> p (c d)")
                )
                st["xT"] = xT

            def mm1_exp(b: int, c: int):
                st = state[b]
                if c == 0:
                    o0 = accpool.tile([128, 512], F32, tag="outT")
                    o1 = accpool.tile([128, 512], F32, tag="outT")
                    st["outT"] = [o0, o1]
                    st["a_tiles"] = {}
                xT, bias_col = st["xT"], st["bias_col"]
                g = gpool.tile([128, B], F32)
                for h in range(2):
                    nc.tensor.matmul(
                        g[:, h * 512:(h + 1) * 512],
                        lhsT=xT[:, c * 128:(c + 1) * 128],
                        rhs=xT[:, h * 512:(h + 1) * 512],
                        start=True, stop=True,
                    )
                a_c = apool.tile([128, B], BF16)
                nc.scalar.activation(
                    a_c[:], g[:], EXP,
                    bias=bias_col[:, c:c + 1], scale=G_SCALE,
                )
                # zero the diagonal of the c-th 128x128 sub-block
                nc.vector.tensor_mul(
                    a_c[:, c * 128:(c + 1) * 128],
                    a_c[:, c * 128:(c + 1) * 128],
                    maskbar[:],
                )
                st["a_tiles"][c] = a_c

            def mm2(b: int, c: int):
                st = state[b]
                a_c = st["a_tiles"].pop(c)
                for h in range(2):
                    nc.tensor.matmul(
                        st["outT"][h][:],
                        lhsT=st["xr"][:, c, :],
                        rhs=a_c[:, h * 512:(h + 1) * 512],
                        start=(c == 0), stop=(c == C - 1),
                    )

            def casts(b: int):
                st = state[b]
                outT_sb = otpool.tile([128, B], BF16)
                for h in range(2):
                    nc.vector.tensor_copy(
                        outT_sb[:, h * 512:(h + 1) * 512], st["outT"][h][:]
                    )
                st["outT_sb"] = outT_sb

            def trans_out(b: int, c: int):
                st = state[b]
                if c == 0:
                    trt2 = trpool.tile([128, C, D], BF16, tag="trt")
                    st["trt2"] = trt2
                nc.tensor.transpose(
                    out=st["trt2"][:, c, :],
                    in_=st["outT_sb"][:, c * 128:(c + 1) * 128],
                    identity=identb[:],
                )

            def tail(b: int):
                st = state.pop(b)
                tmp = tmppool.tile([128, C, D], F32)
                nc.vector.tensor_mul(
                    tmp[:], st["trt2"][:],
                    st["escale"][:].unsqueeze(2).broadcast_to([128, C, D]),
                )
                out_final = opool.tile([128, C, D], F32)
                nc.vector.scalar_tensor_tensor(
                    out=out_final[:], in0=st["x_sb"][:], scalar=1.0 / B,
                    in1=tmp[:], op0=MULT, op1=ADD,
                )
                nc.sync.dma_start(out=fout_v[b], in_=out_final[:])

            # 3-deep software pipeline: while block b's matmul loop runs,
            # block b+1's in-transposes and block b-1's out-transposes fill
            # the PE gaps left by waiting on ScalarE's exp.
            stage_load(0)
            if nb > 1:
                stage_load(1)
            for c in range(C):
                trans_in(0, c)
            xt_copy(0)
            for b in range(nb):
                if b >= 1:
                    casts(b - 1)
                for c in range(C + LAG):
                    if c < C:
                        mm1_exp(b, c)
                        if b + 1 < nb:
                            trans_in(b + 1, c)
                        if b >= 1:
                            trans_out(b - 1, c)
                    if c >= LAG:
                        mm2(b, c - LAG)
                if b + 1 < nb:
                    xt_copy(b + 1)
                if b >= 1:
                    tail(b - 1)
                if b + 2 < nb:
                    stage_load(b + 2)
            casts(nb - 1)
            for c in range(C):
                trans_out(nb - 1, c)
            tail(nb - 1)

    nc.compile()
    return nc


_CACHE: dict[int, bacc.Bacc] = {}


def _get_nc(nb: int = NB_FULL) -> bacc.Bacc:
    if nb not in _CACHE:
        _CACHE[nb] = build(nb)
    return _CACHE[nb]


def run(features: np.ndarray, nc: bacc.Bacc | None = None, **spmd_kwargs):
    """Shard rows across 8 cores, run, gather. Returns (out, BassKernelResults)."""
    features = np.ascontiguousarray(features, dtype=np.float32)
    assert features.shape == (N_TOTAL, D)
    if nc is None:
        nc = _get_nc()
    core_ids = list(range(NCORES))
    shards = np.split(features, NCORES, axis=0)
    in_maps = [{"features": s} for s in shards]
    res = run_bass_kernel_spmd(nc, in_maps, core_ids, **spmd_kwargs)
    out = np.concatenate([res.results[i]["out"] for i in range(NCORES)], axis=0)
    return out, res


def kernel(features: np.ndarray) -> np.ndarray:
    out, _ = run(features)
    return out
